# revision 1
# baseline (speedup 1.0000x reference)
"""CapsNet routing-by-agreement kernel for 8 TRN2 NeuronCores.

Strategy (in_caps sharded 8-way):
  - Each core owns I_loc = 512 in_caps. Its W shard lives entirely in SBUF
    (two bf16 layouts, 8 MB), so routing iterations do ZERO HBM traffic for
    W / u_hat.  u_hat is never materialized; each routing iteration
    recomputes the two W contractions on the TensorEngine with 128-deep
    packed contractions:
      a-path:  Wv[b,i,o,k] = sum_j W[i,o,j,k] v[b,o,j]   (contract (o8,j)=128,
               block-diag v as stationary operand)
               a[b,i,o]    = sum_k u[b,i,k] Wv[b,i,o,k]  (DVE mul + add-tree)
      s-path:  s[b,o,j]    = sum_{i,k} (c*u)[..] W[..]   (contract i mod 128,
               PSUM-accum over (i-block, k); 8x block-diag fanout over o8,
               diagonal extracted with a constant mask)
  - The only cross-core data is the per-out-capsule sum s [64,32,16]:
    bf16 AllGather (64 KB per rank) + local on-chip sum, once per routing
    iteration (3x).
  - All layout permutations of the inputs are done host-side in numpy; every
    on-chip tensor is DMA'd contiguously.

Index conventions (per core c): i_glob = c*512 + Gq*128 + p (Gq in 0..3,
p in 0..127);  o = oc*8 + o8 (oc in 0..3);  q = o8*16 + j (j in 0..15).

Host layouts:
  wd  [128,16384] bf16 : wd[q, ((oc*4+Gq)*8+k)*128 + p]  = W[i,o,j,k]
  wb  [128,16384] bf16 : wb[p, ((Gq*8+k)*4+oc)*128 + q]  = W[i,o,j,k]
  uy  [128, 2048] bf16 : uy[p, (Gq*8+k)*64 + b]          = u[b,i,k]
  dlt [128,  128] bf16 : dlt[q, m] = (q//16 == m//16)      (squash sum_j)
  mbd [128, 2048] bf16 : mbd[q, (oc,o8p,b)] = (q//16 == o8p)
  msk [128,  512] f32  : msk[q, (o8p,b)]   = (q//16 == o8p)
Output:
  vout [128, 256] f32 : vout[o8*16+j, oc*64+b] = v[b, oc*8+o8, j]
"""

import os
import sys

import numpy as np
import ml_dtypes

sys.path.insert(0, "/opt/trn_rl_repo")

B, IN_CAPS, IN_DIM = 64, 4096, 8
OUT_CAPS, OUT_DIM = 32, 16
EPS = 1e-8
N_CORES = 8
I_LOC = IN_CAPS // N_CORES  # 512
NG = I_LOC // 128           # 4

_BF16 = ml_dtypes.bfloat16

_CACHE = {}


def _build_program(reps=1, variant="full"):
    import concourse.bass as bass
    import concourse.bacc as bacc
    import concourse.mybir as mybir
    import concourse.tile as tile
    from contextlib import ExitStack

    f32 = mybir.dt.float32
    bf16 = mybir.dt.bfloat16
    vset = set(variant.split(","))
    exch = ("ag" if "ag" in vset else
            "ar" if "ar" in vset else
            "noexch" if "noexch" in vset else "ag")
    AF = mybir.ActivationFunctionType
    ALU = mybir.AluOpType

    # Pin Exp+Ln to the single combined table set so the table-load pass
    # emits one load instead of thrashing between exp/ln sets each squash.
    import concourse.hw_specs as hw_specs
    if not getattr(bacc, "_caps_tables_patched", False):
        _orig_tables = hw_specs.get_activation_tables

        def _patched_tables(arch):
            tabs = dict(_orig_tables(arch))
            AFt = mybir.ActivationFunctionType
            out = {}
            for name, funcs in tabs.items():
                if name != "natural_log_exp_and_others":
                    funcs = funcs - {AFt.Exp, AFt.Ln}
                out[name] = funcs
            return out

        # NOTE: tried pinning Exp/Ln to the combined table set here to kill
        # table-load thrash, but remapping get_activation_tables corrupts
        # activation numerics (walrus act.json remap) - left disabled.
        bacc._caps_tables_patched = True

    nc = bacc.Bacc(
        "TRN2",
        target_bir_lowering=False,
        debug=False,
        enable_asserts=False,
        num_devices=N_CORES,
    )

    wd_d = nc.dram_tensor("wd", [128, 16384], bf16, kind="ExternalInput")
    wb_d = nc.dram_tensor("wb", [128, 16384], bf16, kind="ExternalInput")
    uy_d = nc.dram_tensor("uy", [128, 2048], bf16, kind="ExternalInput")
    dlt_d = nc.dram_tensor("dlt", [128, 128], bf16, kind="ExternalInput")
    mbd_d = nc.dram_tensor("mbd", [128, 2048], bf16, kind="ExternalInput")
    msk_d = nc.dram_tensor("msk", [128, 512], f32, kind="ExternalInput")
    vout_d = nc.dram_tensor("vout", [128, 256], f32, kind="ExternalOutput")

    with tile.TileContext(nc) as tc:
        with ExitStack() as ctx:
            sb = ctx.enter_context(tc.tile_pool(name="sb", bufs=1))
            ps_pool_s = ctx.enter_context(
                tc.tile_pool(name="ps_s", bufs=1, space="PSUM"))
            ps_pool_wv = ctx.enter_context(
                tc.tile_pool(name="ps_wv", bufs=2, space="PSUM"))
            dram = ctx.enter_context(tc.tile_pool(name="dram", bufs=2,
                                                  space="DRAM"))

            WD = sb.tile([128, 16384], bf16, tag="WD")
            WBH = [sb.tile([128, 8192], bf16, tag=f"WB{i}", name=f"WB{i}")
                   for i in range(2)]
            UY = sb.tile([128, 2048], bf16, tag="UY")
            DLT = sb.tile([128, 128], bf16, tag="DLT")
            MBD = sb.tile([128, 2048], bf16, tag="MBD")
            MSK = sb.tile([128, 512], f32, tag="MSK")

            nc.sync.dma_start(WBH[0][:], wb_d[:, 0:8192])
            nc.sync.dma_start(UY[:], uy_d[:])
            nc.sync.dma_start(WBH[1][:], wb_d[:, 8192:16384])
            nc.sync.dma_start(WD[:], wd_d[:])
            nc.sync.dma_start(DLT[:], dlt_d[:])
            nc.sync.dma_start(MBD[:], mbd_d[:])
            nc.sync.dma_start(MSK[:], msk_d[:])

            def WBs(idx):
                half, off = divmod(idx * 128, 8192)
                return WBH[half][:, off : off + 128]

            # big per-G scratch, parity double-buffered:
            # wv (drains) -> uwv (in-place mul) -> tree partials -> cu
            WVP = [sb.tile([128, 16384], bf16, tag=f"WV{i}", name=f"WV{i}")
                   for i in range(2)]
            BLOG = sb.tile([128, 8192], bf16, tag="BLOG")
            AG = sb.tile([128, 2048], bf16, tag="AG")
            EG = sb.tile([128, 2048], bf16, tag="EG")
            URC = sb.tile([128, 512], bf16, tag="URC")
            SMT = sb.tile([128, 1024], bf16, tag="SMT")
            MSKD = sb.tile([128, 512], f32, tag="MSKD")
            GAT = sb.tile([128, 2048], bf16, tag="GAT")
            SSB = sb.tile([128, 256], f32, tag="SSB")
            SSBH = sb.tile([128, 256], bf16, tag="SSBH")
            SE = sb.tile([128, 64], f32, tag="SE")
            RCP = sb.tile([128, 64], f32, tag="RCP")
            RCPB = sb.tile([128, 64], bf16, tag="RCPB")
            VBD = sb.tile([128, 2048], bf16, tag="VBD")
            S2 = sb.tile([128, 256], bf16, tag="S2")
            SSP1 = sb.tile([128, 256], f32, tag="SSP1")
            RCP1 = sb.tile([128, 256], f32, tag="RCP1")
            LNV = sb.tile([128, 256], f32, tag="LNV")
            RSQ = sb.tile([128, 256], f32, tag="RSQ")
            T1 = sb.tile([128, 256], f32, tag="T1")
            SCL = sb.tile([128, 256], f32, tag="SCL")
            VSB = sb.tile([128, 256], bf16, tag="VSB")
            VSF = sb.tile([128, 256], f32, tag="VSF")

            def squash(t, rep):
                """SSB = this core's partial s; exchange + squash -> v."""
                if exch == "ag":
                    nc.vector.tensor_copy(SSBH[:], SSB[:])
                    ag_in = dram.tile([128, 256], bf16, tag="ag_in",
                                      name=f"ag_in_{rep}_{t}")
                    ag_out = dram.tile([1024, 256], bf16, tag="ag_out",
                                       name=f"ag_out_{rep}_{t}")
                    nc.sync.dma_start(ag_in[:], SSBH[:])
                    nc.gpsimd.collective_compute(
                        "AllGather", ALU.bypass,
                        replica_groups=[list(range(N_CORES))],
                        ins=[ag_in[:].opt()], outs=[ag_out[:].opt()],
                    )
                    nc.sync.dma_start(
                        GAT[:].rearrange("p (r f) -> p r f", r=8),
                        ag_out[:].rearrange("(r p) f -> p r f", p=128),
                    )
                    g3 = GAT[:].rearrange("p (r f) -> p r f", r=8)
                    nc.vector.tensor_add(g3[:, 0:4], g3[:, 0:4], g3[:, 4:8])
                    nc.vector.tensor_add(g3[:, 0:2], g3[:, 0:2], g3[:, 2:4])
                    nc.vector.tensor_add(SSB[:], GAT[:, 0:256], GAT[:, 256:512])
                elif exch == "ar":
                    ar_in = dram.tile([128, 256], f32, tag="ar_in",
                                      name=f"ar_in_{rep}_{t}")
                    ar_out = dram.tile([128, 256], f32, tag="ar_out",
                                       name=f"ar_out_{rep}_{t}")
                    nc.gpsimd.dma_start(ar_in[:], SSB[:])
                    nc.gpsimd.collective_compute(
                        "AllReduce", ALU.add,
                        replica_groups=[list(range(N_CORES))],
                        ins=[ar_in[:].opt()], outs=[ar_out[:].opt()],
                    )
                    nc.gpsimd.dma_start(SSB[:], ar_out[:])
                elif exch == "noexch":
                    pass  # timing-only: skip the cross-core exchange
                # ss = sum_j s^2 (dup'd over 16 via DLT matmul)
                nc.vector.tensor_mul(S2[:], SSB[:], SSB[:])
                ps_sq = ps_pool_wv.tile([128, 256], f32, tag="wvp",
                                        name=f"ps_sq_{rep}_{t}")
                nc.tensor.matmul(ps_sq[:], DLT[:], S2[:], start=True, stop=True)
                # scale = ss/(1+ss)/sqrt(ss+eps);  rsqrt via exp(-0.5*ln)
                nc.vector.tensor_scalar_add(SSP1[:], ps_sq[:], 1.0)
                nc.vector.reciprocal(RCP1[:], SSP1[:])
                nc.vector.tensor_scalar_add(LNV[:], ps_sq[:], EPS)
                nc.scalar.activation(LNV[:], LNV[:], AF.Ln)
                nc.scalar.activation(RSQ[:], LNV[:], AF.Exp, scale=-0.5)
                nc.vector.tensor_mul(T1[:], RCP1[:], RSQ[:])
                nc.vector.tensor_mul(SCL[:], ps_sq[:], T1[:])
                if t < 3:
                    nc.vector.tensor_mul(VSB[:], SSB[:], SCL[:])
                    vsb_b = (VSB[:].rearrange("q (oc b) -> q oc b", oc=4)
                             .unsqueeze(2).broadcast_to([128, 4, 8, 64]))
                    mbd4 = MBD[:].rearrange("q (oc o8 b) -> q oc o8 b",
                                            oc=4, o8=8)
                    vbd4 = VBD[:].rearrange("q (oc o8 b) -> q oc o8 b",
                                            oc=4, o8=8)
                    nc.vector.tensor_mul(vbd4, vsb_b, mbd4)
                else:
                    nc.vector.tensor_mul(VSF[:], SSB[:], SCL[:])
                    nc.sync.dma_start(vout_d[:], VSF[:])

            for rep in range(reps):
                # ---------- iteration 1: s1 = (1/32) sum_i u_hat ----------
                ps_s1 = ps_pool_s.tile([128, 2048], f32, tag="ps_s",
                                       name=f"ps_s0_{rep}")
                for Gq in range(NG):
                    for k in range(8):
                        for oc in range(4):
                            nc.tensor.matmul(
                                ps_s1[:, oc * 512 : oc * 512 + 64],
                                WBs((Gq * 8 + k) * 4 + oc),
                                UY[:, (Gq * 8 + k) * 64 :
                                      (Gq * 8 + k) * 64 + 64],
                                start=(Gq == 0 and k == 0),
                                stop=(Gq == NG - 1 and k == 7),
                            )
                for oc in range(4):
                    nc.scalar.mul(SSB[:, oc * 64 : oc * 64 + 64],
                                  ps_s1[:, oc * 512 : oc * 512 + 64],
                                  1.0 / 32.0)
                squash(1, rep)

                # ---------- iterations 2, 3 ----------
                for t in (2, 3):
                    ps_s = ps_pool_s.tile([128, 2048], f32, tag="ps_s",
                                          name=f"ps_s{rep}_{t}")

                    def wv_phase(Gq):
                        WVG = WVP[Gq % 2]
                        for k in range(8):
                            for ocp in range(2) if "skipwv" not in vset else []:
                                wvp = ps_pool_wv.tile(
                                    [128, 1024], f32, tag="wvp",
                                    name=f"wvp_{rep}_{t}_{Gq}_{k}_{ocp}")
                                for kk in range(2):
                                    oc = ocp * 2 + kk
                                    nc.tensor.matmul(
                                        wvp[:, kk * 512 : kk * 512 + 512],
                                        WD[:, ((oc * 4 + Gq) * 8 + k) * 128 :
                                              ((oc * 4 + Gq) * 8 + k) * 128
                                              + 128],
                                        VBD[:, oc * 512 : oc * 512 + 512],
                                        start=True, stop=True,
                                    )
                                # contiguous drain of the oc-pair; at the
                                # iteration-restart ramp (Gq==0) DVE is idle,
                                # so split drains between ACT and DVE
                                dst = WVG[:, k * 2048 + ocp * 1024 :
                                             k * 2048 + ocp * 1024 + 1024]
                                if Gq == 0 and (k % 2 == ocp):
                                    nc.vector.tensor_copy(dst, wvp[:])
                                else:
                                    nc.scalar.copy(dst, wvp[:])

                    def post_a(Gq):
                        WVG = WVP[Gq % 2]
                        wvg4 = WVG[:].rearrange("p (k x b) -> p k x b",
                                                k=8, x=32)
                        uyg4 = (UY[:, Gq * 512 : Gq * 512 + 512]
                                .rearrange("p (k b) -> p k b", k=8)
                                .unsqueeze(2).broadcast_to([128, 8, 32, 64]))
                        if "skipmt" not in vset:
                            nc.vector.tensor_mul(wvg4, wvg4, uyg4)
                            nc.vector.tensor_add(WVG[:, 0:8192],
                                                 WVG[:, 0:8192],
                                                 WVG[:, 8192:16384])
                            nc.vector.tensor_add(WVG[:, 8192:12288],
                                                 WVG[:, 0:4096],
                                                 WVG[:, 4096:8192])
                            nc.vector.tensor_add(AG[:], WVG[:, 8192:10240],
                                                 WVG[:, 10240:12288])
                        gsl = slice(Gq * 2048, Gq * 2048 + 2048)
                        if t == 2:
                            nc.vector.tensor_copy(BLOG[:, gsl], AG[:])
                        else:
                            nc.vector.tensor_add(AG[:], BLOG[:, gsl], AG[:])
                        nc.scalar.activation(EG[:], AG[:], AF.Exp)

                    def post_b(Gq):
                        WVG = WVP[Gq % 2]
                        wvg4 = WVG[:].rearrange("p (k x b) -> p k x b",
                                                k=8, x=32)
                        uyg4 = (UY[:, Gq * 512 : Gq * 512 + 512]
                                .rearrange("p (k b) -> p k b", k=8)
                                .unsqueeze(2).broadcast_to([128, 8, 32, 64]))
                        # sum over o: contiguous halving tree (o is outer)
                        nc.vector.tensor_add(SMT[:], EG[:, 0:1024],
                                             EG[:, 1024:2048])
                        nc.vector.tensor_add(SMT[:, 0:512], SMT[:, 0:512],
                                             SMT[:, 512:1024])
                        nc.vector.tensor_add(SMT[:, 0:256], SMT[:, 0:256],
                                             SMT[:, 256:512])
                        nc.vector.tensor_add(SMT[:, 0:128], SMT[:, 0:128],
                                             SMT[:, 128:256])
                        nc.vector.tensor_add(SE[:], SMT[:, 0:64],
                                             SMT[:, 64:128])
                        nc.vector.reciprocal(RCP[:], SE[:])
                        nc.scalar.copy(RCPB[:], RCP[:])
                        # urc = u * (1/Z): folds softmax denom into cu
                        nc.vector.tensor_mul(
                            URC[:].rearrange("p (k b) -> p k b", k=8),
                            UY[:, Gq * 512 : Gq * 512 + 512]
                               .rearrange("p (k b) -> p k b", k=8),
                            RCPB[:].unsqueeze(1).broadcast_to([128, 8, 64]))
                        # cu = e * urc (into WVG, now dead)
                        egb4 = (EG[:].rearrange("p (x b) -> p x b", x=32)
                                .unsqueeze(1).broadcast_to([128, 8, 32, 64]))
                        urc4 = (URC[:].rearrange("p (k b) -> p k b", k=8)
                                .unsqueeze(2).broadcast_to([128, 8, 32, 64]))
                        if "skipcu" not in vset:
                            nc.vector.tensor_mul(wvg4, egb4, urc4)
                        for k in range(8) if "skipsmm" not in vset else []:
                            for oc in range(4):
                                nc.tensor.matmul(
                                    ps_s[:, oc * 512 : oc * 512 + 512],
                                    WBs((Gq * 8 + k) * 4 + oc),
                                    WVG[:, k * 2048 + oc * 512 :
                                           k * 2048 + oc * 512 + 512],
                                    start=(Gq == 0 and k == 0),
                                    stop=(Gq == NG - 1 and k == 7),
                                )

                    # software pipeline: split post around exp so the ACT
                    # stream is [exp(G-1), drains(G), ...] and DVE never waits
                    # for a drain batch to reach its exp
                    for Gq in range(NG + 1):
                        if Gq >= 1:
                            post_a(Gq - 1)
                        if Gq < NG:
                            wv_phase(Gq)
                        if Gq >= 1:
                            post_b(Gq - 1)
                    # fused diag extract via mask
                    if "skipsmm" not in vset:
                        mm4 = MSKD[:].rearrange("q (o8 b) -> q o8 b", o8=8)
                        for oc in range(4):
                            nc.vector.tensor_mul(
                                MSKD[:], ps_s[:, oc * 512 : oc * 512 + 512],
                                MSK[:])
                            nc.vector.tensor_reduce(
                                SSB[:, oc * 64 : oc * 64 + 64],
                                MSKD[:].rearrange("q (o8 b) -> q b o8", o8=8),
                                axis=mybir.AxisListType.X, op=ALU.add)
                    squash(t, rep)

    nc.compile()
    return nc


def _host_prep(u, W):
    """Build per-core input maps (all host-side permutes)."""
    in_maps = []
    q = np.arange(128)
    dlt = (q[:, None] // 16 == q[None, :] // 16).astype(_BF16)
    o8p = np.arange(8)
    diag = (q[:, None] // 16 == o8p[None, :])
    mbd = np.ascontiguousarray(
        np.broadcast_to(diag[:, None, :, None], (128, 4, 8, 64))
    ).reshape(128, 2048).astype(_BF16)
    msk = np.ascontiguousarray(
        np.broadcast_to(diag[:, :, None], (128, 8, 64))
    ).reshape(128, 512).astype(np.float32)
    for c in range(N_CORES):
        Ws = np.asarray(W[c * I_LOC : (c + 1) * I_LOC], dtype=np.float32)
        us = np.asarray(u[:, c * I_LOC : (c + 1) * I_LOC, :], dtype=np.float32)
        Wr = Ws.reshape(NG, 128, 4, 8, 16, 8)           # [Gq,p,oc,o8,j,k]
        wd = np.ascontiguousarray(
            Wr.transpose(3, 4, 2, 0, 5, 1)              # [o8,j,oc,Gq,k,p]
        ).reshape(128, 16384).astype(_BF16)
        wb = np.ascontiguousarray(
            Wr.transpose(1, 0, 5, 2, 3, 4)              # [p,Gq,k,oc,o8,j]
        ).reshape(128, 16384).astype(_BF16)
        ur = us.reshape(B, NG, 128, 8)                  # [b,Gq,p,k]
        uy = np.ascontiguousarray(
            ur.transpose(2, 1, 3, 0)                    # [p,Gq,k,b]
        ).reshape(128, 2048).astype(_BF16)
        in_maps.append({"wd": wd, "wb": wb, "uy": uy, "dlt": dlt,
                        "mbd": mbd, "msk": msk})
    return in_maps


def kernel(u, W):
    from concourse.bass_utils import run_bass_kernel_spmd

    if "nc" not in _CACHE:
        _CACHE["nc"] = _build_program(variant="ag")
    nc = _CACHE["nc"]

    in_maps = _host_prep(u, W)
    res = run_bass_kernel_spmd(
        nc, in_maps, core_ids=list(range(N_CORES)),
        trace=bool(int(os.environ.get("CAPS_TRACE", "0"))),
    )
    if isinstance(res, tuple):
        results = res[0]
    else:
        _CACHE["last_results"] = res
        results = res.results
    vout = results[0]["vout"]  # [128, 256]; identical on every core
    t = vout.reshape(8, 16, 4, 64)          # [o8, j, oc, b]
    v = np.ascontiguousarray(t.transpose(3, 2, 0, 1)).reshape(B, OUT_CAPS, OUT_DIM)
    return v.astype(np.float32)



# revision 23
# speedup vs baseline: 1.8579x; 1.8579x over previous
"""CapsNet routing-by-agreement kernel for 8 TRN2 NeuronCores.

Strategy (in_caps sharded 8-way):
  - Each core owns I_loc = 512 in_caps. Its W shard lives entirely in SBUF
    (two bf16 layouts, 8 MB), so routing iterations do ZERO HBM traffic for
    W / u_hat.  u_hat is never materialized; each routing iteration
    recomputes the two W contractions on the TensorEngine with 128-deep
    packed contractions:
      a-path:  Wv[b,i,o,k] = sum_j W[i,o,j,k] v[b,o,j]   (contract (o8,j)=128,
               block-diag v as stationary operand)
               a[b,i,o]    = sum_k u[b,i,k] Wv[b,i,o,k]  (DVE mul + add-tree)
      s-path:  s[b,o,j]    = sum_{i,k} (c*u)[..] W[..]   (contract i mod 128,
               PSUM-accum over (i-block, k); 8x block-diag fanout over o8,
               diagonal extracted with a constant mask)
  - The only cross-core data is the per-out-capsule sum s [64,32,16]:
    bf16 AllGather (64 KB per rank) + local on-chip sum, once per routing
    iteration.  The 3rd iteration ships the f32 partial s instead and the
    host does the final 8-way reduce + squash (saves one collective).
  - Engine schedule (iters 2-3): ACT drains all wv PSUM->SBUF (copy is in
    every act table set, so no table thrash); DVE runs per-k uwv muls that
    chase the drains plus the k-reduction tree; the cu=e*urc mul is split
    DVE 3/8 : Pool 5/8 (Pool must never touch PSUM - HW constraint).
  - All layout permutations of the inputs are done host-side in numpy; every
    on-chip tensor is DMA'd contiguously.

Index conventions (per core c): i_glob = c*512 + Gq*128 + p (Gq in 0..3,
p in 0..127);  o = oc*8 + o8 (oc in 0..3);  q = o8*16 + j (j in 0..15).

Host layouts:
  wd  [128,16384] bf16 : wd[q, ((oc*4+Gq)*8+k)*128 + p]  = W[i,o,j,k]
  wb  [128,16384] bf16 : wb[p, ((Gq*8+k)*4+oc)*128 + q]  = W[i,o,j,k]
  uy  [128, 2048] bf16 : uy[p, (Gq*8+k)*64 + b]          = u[b,i,k]
  dlt [128,  128] bf16 : dlt[q, m] = (q//16 == m//16)      (squash sum_j)
  mbd [128, 2048] bf16 : mbd[q, (oc,o8p,b)] = (q//16 == o8p)
  msk [128,  512] f32  : msk[q, (o8p,b)]   = (q//16 == o8p)
Output:
  vout [128, 256] f32 : vout[o8*16+j, oc*64+b] = v[b, oc*8+o8, j]
"""

import os
import sys

import numpy as np
import ml_dtypes

sys.path.insert(0, "/opt/trn_rl_repo")

B, IN_CAPS, IN_DIM = 64, 4096, 8
OUT_CAPS, OUT_DIM = 32, 16
EPS = 1e-8
N_CORES = 8
I_LOC = IN_CAPS // N_CORES  # 512
NG = I_LOC // 128           # 4

_BF16 = ml_dtypes.bfloat16

_CACHE = {}


DEFAULT_VARIANT = "ag,rebal,hostsq"


def _build_program(reps=1, variant=DEFAULT_VARIANT):
    import concourse.bass as bass
    import concourse.bacc as bacc
    import concourse.mybir as mybir
    import concourse.tile as tile
    from contextlib import ExitStack

    f32 = mybir.dt.float32
    bf16 = mybir.dt.bfloat16
    vset = set(variant.split(","))
    exch = ("ag" if "ag" in vset else
            "ar" if "ar" in vset else
            "noexch" if "noexch" in vset else "ag")
    rebal = "rebal" in vset     # spread elementwise work over ACT/DVE/Pool
    hostsq = "hostsq" in vset   # final reduce+squash on host (no 3rd exchange)
    AF = mybir.ActivationFunctionType
    ALU = mybir.AluOpType

    # Pin Exp+Ln to the single combined table set so the table-load pass
    # emits one load instead of thrashing between exp/ln sets each squash.
    import concourse.hw_specs as hw_specs
    if not getattr(bacc, "_caps_tables_patched", False):
        _orig_tables = hw_specs.get_activation_tables

        def _patched_tables(arch):
            tabs = dict(_orig_tables(arch))
            AFt = mybir.ActivationFunctionType
            out = {}
            for name, funcs in tabs.items():
                if name != "natural_log_exp_and_others":
                    funcs = funcs - {AFt.Exp, AFt.Ln}
                out[name] = funcs
            return out

        # NOTE: tried pinning Exp/Ln to the combined table set here to kill
        # table-load thrash, but remapping get_activation_tables corrupts
        # activation numerics (walrus act.json remap) - left disabled.
        bacc._caps_tables_patched = True

    nc = bacc.Bacc(
        "TRN2",
        target_bir_lowering=False,
        debug=False,
        enable_asserts=False,
        num_devices=N_CORES,
    )

    wd_d = nc.dram_tensor("wd", [128, 16384], bf16, kind="ExternalInput")
    wb_d = nc.dram_tensor("wb", [128, 16384], bf16, kind="ExternalInput")
    uy_d = nc.dram_tensor("uy", [128, 2048], bf16, kind="ExternalInput")
    dlt_d = nc.dram_tensor("dlt", [128, 128], bf16, kind="ExternalInput")
    mbd_d = nc.dram_tensor("mbd", [128, 2048], bf16, kind="ExternalInput")
    msk_d = nc.dram_tensor("msk", [128, 512], f32, kind="ExternalInput")
    vout_d = nc.dram_tensor("vout", [128, 256], f32, kind="ExternalOutput")

    with tile.TileContext(nc) as tc:
        with ExitStack() as ctx:
            sb = ctx.enter_context(tc.tile_pool(name="sb", bufs=1))
            ps_pool_s = ctx.enter_context(
                tc.tile_pool(name="ps_s", bufs=1, space="PSUM"))
            ps_pool_wv = ctx.enter_context(
                tc.tile_pool(name="ps_wv", bufs=2, space="PSUM"))
            dram = ctx.enter_context(tc.tile_pool(name="dram", bufs=2,
                                                  space="DRAM"))

            WD = sb.tile([128, 16384], bf16, tag="WD")
            WBH = [sb.tile([128, 8192], bf16, tag=f"WB{i}", name=f"WB{i}")
                   for i in range(2)]
            UY = sb.tile([128, 2048], bf16, tag="UY")
            DLT = sb.tile([128, 128], bf16, tag="DLT")
            MBD = sb.tile([128, 2048], bf16, tag="MBD")
            MSK = sb.tile([128, 512], f32, tag="MSK")

            if rebal:
                # spread the startup loads over three hwdge queues so the
                # s1 inputs (WB halves + UY) land ~3x sooner
                nc.sync.dma_start(WBH[0][:], wb_d[:, 0:8192])
                nc.scalar.dma_start(WBH[1][:], wb_d[:, 8192:16384])
                nc.gpsimd.dma_start(UY[:], uy_d[:])
                nc.gpsimd.dma_start(DLT[:], dlt_d[:])
                nc.sync.dma_start(WD[:], wd_d[:])
                nc.scalar.dma_start(MBD[:], mbd_d[:])
                nc.gpsimd.dma_start(MSK[:], msk_d[:])
            else:
                nc.sync.dma_start(WBH[0][:], wb_d[:, 0:8192])
                nc.sync.dma_start(UY[:], uy_d[:])
                nc.sync.dma_start(WBH[1][:], wb_d[:, 8192:16384])
                nc.sync.dma_start(WD[:], wd_d[:])
                nc.sync.dma_start(DLT[:], dlt_d[:])
                nc.sync.dma_start(MBD[:], mbd_d[:])
                nc.sync.dma_start(MSK[:], msk_d[:])

            def WBs(idx):
                half, off = divmod(idx * 128, 8192)
                return WBH[half][:, off : off + 128]

            # big per-G scratch, parity double-buffered:
            # wv (drains) -> uwv (in-place mul) -> tree partials -> cu
            WVP = [sb.tile([128, 16384], bf16, tag=f"WV{i}", name=f"WV{i}")
                   for i in range(2)]
            BLOG = sb.tile([128, 8192], bf16, tag="BLOG")
            AG = sb.tile([128, 2048], bf16, tag="AG")
            EG = sb.tile([128, 2048], bf16, tag="EG")
            URC = sb.tile([128, 512], bf16, tag="URC")
            SMT = sb.tile([128, 1024], bf16, tag="SMT")
            MSKD = sb.tile([128, 512], f32, tag="MSKD")
            MSKD2 = sb.tile([128, 512], f32, tag="MSKD2")
            GAT = sb.tile([128, 2048], bf16, tag="GAT")
            SSB = sb.tile([128, 256], f32, tag="SSB")
            SSBH = sb.tile([128, 256], bf16, tag="SSBH")
            SE = sb.tile([128, 64], f32, tag="SE")
            RCP = sb.tile([128, 64], f32, tag="RCP")
            RCPB = sb.tile([128, 64], bf16, tag="RCPB")
            VBD = sb.tile([128, 2048], bf16, tag="VBD")
            S2 = sb.tile([128, 256], bf16, tag="S2")
            SSP1 = sb.tile([128, 256], f32, tag="SSP1")
            RCP1 = sb.tile([128, 256], f32, tag="RCP1")
            LNV = sb.tile([128, 256], f32, tag="LNV")
            RSQ = sb.tile([128, 256], f32, tag="RSQ")
            T1 = sb.tile([128, 256], f32, tag="T1")
            SCL = sb.tile([128, 256], f32, tag="SCL")
            VSB = sb.tile([128, 256], bf16, tag="VSB")
            VSF = sb.tile([128, 256], f32, tag="VSF")

            def squash(t, rep):
                """SSB = this core's partial s; exchange + squash -> v."""
                if t == 3 and hostsq:
                    # ship the local f32 partial; host sums 8 partials and
                    # squashes (drops the 3rd collective + squash chain)
                    nc.sync.dma_start(vout_d[:], SSB[:])
                    return
                if exch == "ag":
                    nc.vector.tensor_copy(SSBH[:], SSB[:])
                    ag_in = dram.tile([128, 256], bf16, tag="ag_in",
                                      name=f"ag_in_{rep}_{t}")
                    ag_out = dram.tile([1024, 256], bf16, tag="ag_out",
                                       name=f"ag_out_{rep}_{t}")
                    nc.sync.dma_start(ag_in[:], SSBH[:])
                    nc.gpsimd.collective_compute(
                        "AllGather", ALU.bypass,
                        replica_groups=[list(range(N_CORES))],
                        ins=[ag_in[:].opt()], outs=[ag_out[:].opt()],
                    )
                    g3v = GAT[:].rearrange("p (r f) -> p r f", r=8)
                    agv = ag_out[:].rearrange("(r p) f -> p r f", p=128)
                    if rebal:
                        # split the gather readback over two hwdge queues
                        nc.sync.dma_start(g3v[:, 0:4], agv[:, 0:4])
                        nc.scalar.dma_start(g3v[:, 4:8], agv[:, 4:8])
                    else:
                        nc.sync.dma_start(g3v, agv)
                    g3 = GAT[:].rearrange("p (r f) -> p r f", r=8)
                    nc.vector.tensor_add(g3[:, 0:4], g3[:, 0:4], g3[:, 4:8])
                    nc.vector.tensor_add(g3[:, 0:2], g3[:, 0:2], g3[:, 2:4])
                    nc.vector.tensor_add(SSB[:], GAT[:, 0:256], GAT[:, 256:512])
                elif exch == "ar":
                    ar_in = dram.tile([128, 256], f32, tag="ar_in",
                                      name=f"ar_in_{rep}_{t}")
                    ar_out = dram.tile([128, 256], f32, tag="ar_out",
                                       name=f"ar_out_{rep}_{t}")
                    nc.gpsimd.dma_start(ar_in[:], SSB[:])
                    nc.gpsimd.collective_compute(
                        "AllReduce", ALU.add,
                        replica_groups=[list(range(N_CORES))],
                        ins=[ar_in[:].opt()], outs=[ar_out[:].opt()],
                    )
                    nc.gpsimd.dma_start(SSB[:], ar_out[:])
                elif exch == "noexch":
                    pass  # timing-only: skip the cross-core exchange
                # ss = sum_j s^2 (dup'd over 16 via DLT matmul)
                nc.vector.tensor_mul(S2[:], SSB[:], SSB[:])
                ps_sq = ps_pool_wv.tile([128, 256], f32, tag="wvp",
                                        name=f"ps_sq_{rep}_{t}")
                nc.tensor.matmul(ps_sq[:], DLT[:], S2[:], start=True, stop=True)
                # scale = ss/(1+ss)/sqrt(ss+eps);  rsqrt via exp(-0.5*ln)
                nc.vector.tensor_scalar_add(SSP1[:], ps_sq[:], 1.0)
                nc.vector.reciprocal(RCP1[:], SSP1[:])
                nc.vector.tensor_scalar_add(LNV[:], ps_sq[:], EPS)
                nc.scalar.activation(LNV[:], LNV[:], AF.Ln)
                nc.scalar.activation(RSQ[:], LNV[:], AF.Exp, scale=-0.5)
                nc.vector.tensor_mul(T1[:], RCP1[:], RSQ[:])
                nc.vector.tensor_mul(SCL[:], ps_sq[:], T1[:])
                if t < 3:
                    nc.vector.tensor_mul(VSB[:], SSB[:], SCL[:])
                    vsb_b = (VSB[:].rearrange("q (oc b) -> q oc b", oc=4)
                             .unsqueeze(2).broadcast_to([128, 4, 8, 64]))
                    mbd4 = MBD[:].rearrange("q (oc o8 b) -> q oc o8 b",
                                            oc=4, o8=8)
                    vbd4 = VBD[:].rearrange("q (oc o8 b) -> q oc o8 b",
                                            oc=4, o8=8)
                    nc.vector.tensor_mul(vbd4, vsb_b, mbd4)
                else:
                    nc.vector.tensor_mul(VSF[:], SSB[:], SCL[:])
                    nc.sync.dma_start(vout_d[:], VSF[:])

            for rep in range(reps):
                # ---------- iteration 1: s1 = (1/32) sum_i u_hat ----------
                ps_s1 = ps_pool_s.tile([128, 2048], f32, tag="ps_s",
                                       name=f"ps_s0_{rep}")
                for Gq in range(NG):
                    for k in range(8):
                        for oc in range(4):
                            nc.tensor.matmul(
                                ps_s1[:, oc * 512 : oc * 512 + 64],
                                WBs((Gq * 8 + k) * 4 + oc),
                                UY[:, (Gq * 8 + k) * 64 :
                                      (Gq * 8 + k) * 64 + 64],
                                start=(Gq == 0 and k == 0),
                                stop=(Gq == NG - 1 and k == 7),
                            )
                for oc in range(4):
                    nc.scalar.mul(SSB[:, oc * 64 : oc * 64 + 64],
                                  ps_s1[:, oc * 512 : oc * 512 + 64],
                                  1.0 / 32.0)
                squash(1, rep)

                # ---------- iterations 2, 3 ----------
                for t in (2, 3):
                    ps_s = ps_pool_s.tile([128, 2048], f32, tag="ps_s",
                                          name=f"ps_s{rep}_{t}")

                    def wv_phase(Gq):
                        WVG = WVP[Gq % 2]
                        for k in range(8):
                            for ocp in range(2) if "skipwv" not in vset else []:
                                wvp = ps_pool_wv.tile(
                                    [128, 1024], f32, tag="wvp",
                                    name=f"wvp_{rep}_{t}_{Gq}_{k}_{ocp}")
                                for kk in range(2):
                                    oc = ocp * 2 + kk
                                    nc.tensor.matmul(
                                        wvp[:, kk * 512 : kk * 512 + 512],
                                        WD[:, ((oc * 4 + Gq) * 8 + k) * 128 :
                                              ((oc * 4 + Gq) * 8 + k) * 128
                                              + 128],
                                        VBD[:, oc * 512 : oc * 512 + 512],
                                        start=True, stop=True,
                                    )
                                # contiguous drain of the oc-pair; at the
                                # iteration-restart ramp (Gq==0) DVE+Pool are
                                # idle, so split drains three ways there
                                dst = WVG[:, k * 2048 + ocp * 1024 :
                                             k * 2048 + ocp * 1024 + 1024]
                                if rebal:
                                    # NOTE: Pool/GPSIMD cannot read PSUM on
                                    # real HW — drains only on ACT/DVE
                                    if Gq == 0 and (k * 2 + ocp) % 3 == 1:
                                        nc.vector.tensor_copy(dst, wvp[:])
                                    else:
                                        nc.scalar.copy(dst, wvp[:])
                                elif Gq == 0 and (k % 2 == ocp):
                                    nc.vector.tensor_copy(dst, wvp[:])
                                else:
                                    nc.scalar.copy(dst, wvp[:])

                    def post_a(Gq):
                        WVG = WVP[Gq % 2]
                        wvg4 = WVG[:].rearrange("p (k x b) -> p k x b",
                                                k=8, x=32)
                        uyg4 = (UY[:, Gq * 512 : Gq * 512 + 512]
                                .rearrange("p (k b) -> p k b", k=8)
                                .unsqueeze(2).broadcast_to([128, 8, 32, 64]))
                        if "skipmt" in vset:
                            pass
                        elif rebal:
                            # per-k muls chase the drains (DVE starts after
                            # the first k lands instead of after all 16)
                            for k in range(8):
                                nc.vector.tensor_mul(
                                    wvg4[:, k], wvg4[:, k], uyg4[:, k])
                                if k >= 4:
                                    lo = (k - 4) * 2048
                                    hi = k * 2048
                                    nc.vector.tensor_add(
                                        WVG[:, lo : lo + 2048],
                                        WVG[:, lo : lo + 2048],
                                        WVG[:, hi : hi + 2048])
                            nc.vector.tensor_add(WVG[:, 0:2048],
                                                 WVG[:, 0:2048],
                                                 WVG[:, 4096:6144])
                            nc.vector.tensor_add(WVG[:, 2048:4096],
                                                 WVG[:, 2048:4096],
                                                 WVG[:, 6144:8192])
                            nc.vector.tensor_add(AG[:], WVG[:, 0:2048],
                                                 WVG[:, 2048:4096])
                        else:
                            nc.vector.tensor_mul(wvg4, wvg4, uyg4)
                            nc.vector.tensor_add(WVG[:, 0:8192],
                                                 WVG[:, 0:8192],
                                                 WVG[:, 8192:16384])
                            nc.vector.tensor_add(WVG[:, 8192:12288],
                                                 WVG[:, 0:4096],
                                                 WVG[:, 4096:8192])
                            nc.vector.tensor_add(AG[:], WVG[:, 8192:10240],
                                                 WVG[:, 10240:12288])
                        gsl = slice(Gq * 2048, Gq * 2048 + 2048)
                        if t == 2:
                            # BLOG save is off the exp critical path -> Pool
                            (nc.gpsimd if rebal else nc.vector).tensor_copy(
                                BLOG[:, gsl], AG[:])
                        else:
                            nc.vector.tensor_add(AG[:], BLOG[:, gsl], AG[:])
                        nc.scalar.activation(EG[:], AG[:], AF.Exp)

                    def post_b(Gq):
                        WVG = WVP[Gq % 2]
                        wvg4 = WVG[:].rearrange("p (k x b) -> p k x b",
                                                k=8, x=32)
                        uyg4 = (UY[:, Gq * 512 : Gq * 512 + 512]
                                .rearrange("p (k b) -> p k b", k=8)
                                .unsqueeze(2).broadcast_to([128, 8, 32, 64]))
                        # sum over o: contiguous halving tree (o is outer)
                        nc.vector.tensor_add(SMT[:], EG[:, 0:1024],
                                             EG[:, 1024:2048])
                        nc.vector.tensor_add(SMT[:, 0:512], SMT[:, 0:512],
                                             SMT[:, 512:1024])
                        nc.vector.tensor_add(SMT[:, 0:256], SMT[:, 0:256],
                                             SMT[:, 256:512])
                        nc.vector.tensor_add(SMT[:, 0:128], SMT[:, 0:128],
                                             SMT[:, 128:256])
                        nc.vector.tensor_add(SE[:], SMT[:, 0:64],
                                             SMT[:, 64:128])
                        nc.vector.reciprocal(RCP[:], SE[:])
                        nc.scalar.copy(RCPB[:], RCP[:])
                        # urc = u * (1/Z): folds softmax denom into cu
                        nc.vector.tensor_mul(
                            URC[:].rearrange("p (k b) -> p k b", k=8),
                            UY[:, Gq * 512 : Gq * 512 + 512]
                               .rearrange("p (k b) -> p k b", k=8),
                            RCPB[:].unsqueeze(1).broadcast_to([128, 8, 64]))
                        # cu = e * urc (into WVG, now dead)
                        egb4 = (EG[:].rearrange("p (x b) -> p x b", x=32)
                                .unsqueeze(1).broadcast_to([128, 8, 32, 64]))
                        urc4 = (URC[:].rearrange("p (k b) -> p k b", k=8)
                                .unsqueeze(2).broadcast_to([128, 8, 32, 64]))
                        last = Gq == NG - 1
                        if "skipcu" not in vset:
                            if rebal and last:
                                # iteration tail: the s-matmul stop waits on
                                # the final cu, so keep most of it on the
                                # faster DVE (Pool's share would be the tail)
                                nc.vector.tensor_mul(
                                    wvg4[:, 2:5], egb4[:, 2:5], urc4[:, 2:5])
                                nc.gpsimd.tensor_mul(
                                    wvg4[:, 0:2], egb4[:, 0:2], urc4[:, 0:2])
                                nc.vector.tensor_mul(
                                    wvg4[:, 5:8], egb4[:, 5:8], urc4[:, 5:8])
                            elif rebal:
                                # cu = e * urc split DVE/Pool (~3/8 : 5/8 by
                                # engine rates); DVE's k-slice feeds the
                                # first s-matmuls so PE starts sooner
                                nc.vector.tensor_mul(
                                    wvg4[:, 5:8], egb4[:, 5:8], urc4[:, 5:8])
                                nc.gpsimd.tensor_mul(
                                    wvg4[:, 0:3], egb4[:, 0:3], urc4[:, 0:3])
                                nc.gpsimd.tensor_mul(
                                    wvg4[:, 3:5], egb4[:, 3:5], urc4[:, 3:5])
                            else:
                                nc.vector.tensor_mul(wvg4, egb4, urc4)
                        korder = (((2, 3, 4, 5, 6, 7, 0, 1) if last else
                                   (5, 6, 7, 0, 1, 2, 3, 4)) if rebal
                                  else range(8))
                        for ki, k in (enumerate(korder)
                                      if "skipsmm" not in vset else []):
                            for oc in range(4):
                                nc.tensor.matmul(
                                    ps_s[:, oc * 512 : oc * 512 + 512],
                                    WBs((Gq * 8 + k) * 4 + oc),
                                    WVG[:, k * 2048 + oc * 512 :
                                           k * 2048 + oc * 512 + 512],
                                    start=(Gq == 0 and ki == 0),
                                    stop=(Gq == NG - 1 and ki == 7),
                                )

                    # software pipeline: split post around exp so the ACT
                    # stream is [exp(G-1), drains(G), ...] and DVE never waits
                    # for a drain batch to reach its exp
                    for Gq in range(NG + 1):
                        if Gq >= 1:
                            post_a(Gq - 1)
                        if Gq < NG:
                            wv_phase(Gq)
                        if Gq >= 1:
                            post_b(Gq - 1)
                    # fused diag extract via mask (oc pairs split DVE/Pool;
                    # Pool can't reduce along free axes, so halving tree)
                    if "skipsmm" not in vset:
                        for oc in range(4):
                            if rebal and oc >= 2:
                                # DVE does the PSUM read (Pool can't on HW);
                                # Pool sums the SBUF scratch
                                g, scr = nc.gpsimd, MSKD2
                                nc.vector.tensor_mul(
                                    scr[:],
                                    ps_s[:, oc * 512 : oc * 512 + 512],
                                    MSK[:])
                                g.tensor_add(scr[:, 0:256], scr[:, 0:256],
                                             scr[:, 256:512])
                                g.tensor_add(scr[:, 0:128], scr[:, 0:128],
                                             scr[:, 128:256])
                                g.tensor_add(SSB[:, oc * 64 : oc * 64 + 64],
                                             scr[:, 0:64], scr[:, 64:128])
                            else:
                                nc.vector.tensor_mul(
                                    MSKD[:],
                                    ps_s[:, oc * 512 : oc * 512 + 512],
                                    MSK[:])
                                nc.vector.tensor_reduce(
                                    SSB[:, oc * 64 : oc * 64 + 64],
                                    MSKD[:].rearrange(
                                        "q (o8 b) -> q b o8", o8=8),
                                    axis=mybir.AxisListType.X, op=ALU.add)
                    squash(t, rep)

    nc.compile()
    return nc


def _host_prep(u, W):
    """Build per-core input maps (all host-side permutes)."""
    in_maps = []
    q = np.arange(128)
    dlt = (q[:, None] // 16 == q[None, :] // 16).astype(_BF16)
    o8p = np.arange(8)
    diag = (q[:, None] // 16 == o8p[None, :])
    mbd = np.ascontiguousarray(
        np.broadcast_to(diag[:, None, :, None], (128, 4, 8, 64))
    ).reshape(128, 2048).astype(_BF16)
    msk = np.ascontiguousarray(
        np.broadcast_to(diag[:, :, None], (128, 8, 64))
    ).reshape(128, 512).astype(np.float32)
    for c in range(N_CORES):
        Ws = np.asarray(W[c * I_LOC : (c + 1) * I_LOC], dtype=np.float32)
        us = np.asarray(u[:, c * I_LOC : (c + 1) * I_LOC, :], dtype=np.float32)
        Wr = Ws.reshape(NG, 128, 4, 8, 16, 8)           # [Gq,p,oc,o8,j,k]
        wd = np.ascontiguousarray(
            Wr.transpose(3, 4, 2, 0, 5, 1)              # [o8,j,oc,Gq,k,p]
        ).reshape(128, 16384).astype(_BF16)
        wb = np.ascontiguousarray(
            Wr.transpose(1, 0, 5, 2, 3, 4)              # [p,Gq,k,oc,o8,j]
        ).reshape(128, 16384).astype(_BF16)
        ur = us.reshape(B, NG, 128, 8)                  # [b,Gq,p,k]
        uy = np.ascontiguousarray(
            ur.transpose(2, 1, 3, 0)                    # [p,Gq,k,b]
        ).reshape(128, 2048).astype(_BF16)
        in_maps.append({"wd": wd, "wb": wb, "uy": uy, "dlt": dlt,
                        "mbd": mbd, "msk": msk})
    return in_maps


def _postprocess(vouts):
    """vouts: per-core vout [128,256]. With hostsq these are f32 partial s
    sums; reduce over cores and squash here. Layout: [o8*16+j, oc*64+b]."""
    if "hostsq" in DEFAULT_VARIANT.split(","):
        s = np.zeros((128, 256), np.float32)
        for vo in vouts:
            s += np.asarray(vo, np.float32)
        t = s.reshape(8, 16, 4, 64)         # [o8, j, oc, b]
        s = np.ascontiguousarray(t.transpose(3, 2, 0, 1)).reshape(
            B, OUT_CAPS, OUT_DIM)
        sq = np.sum(s * s, axis=-1, keepdims=True)
        v = (sq / (1.0 + sq)) * s / np.sqrt(sq + EPS)
    else:
        t = np.asarray(vouts[0]).reshape(8, 16, 4, 64)
        v = np.ascontiguousarray(t.transpose(3, 2, 0, 1)).reshape(
            B, OUT_CAPS, OUT_DIM)
    return v.astype(np.float32)


def kernel(u, W):
    from concourse.bass_utils import run_bass_kernel_spmd

    if "nc" not in _CACHE:
        _CACHE["nc"] = _build_program()
    nc = _CACHE["nc"]

    in_maps = _host_prep(u, W)
    res = run_bass_kernel_spmd(
        nc, in_maps, core_ids=list(range(N_CORES)),
        trace=bool(int(os.environ.get("CAPS_TRACE", "0"))),
    )
    if isinstance(res, tuple):
        results = res[0]
    else:
        _CACHE["last_results"] = res
        results = res.results
    return _postprocess([r["vout"] for r in results])

